# revision 32
# baseline (speedup 1.0000x reference)
"""Trainium2 Bass kernel for nn_AttnBlock (B=2, C=256, H=W=64, 8 heads, d=32).

Sharding: head-parallel across 8 NeuronCores (core i <-> head i, both batches).
The reference's torch-faithful reshape h.view(B,H,W,C) folds the head dim into
the spatial rows: output rows y in [8i, 8i+8) depend ONLY on head i, so each
core computes its own 8-row output slab and the host just concatenates -- no
collectives needed.

Per-core math (S = H*W = 4096):
  h   = BN(x)                                  [C, S]   bf16
  q   = (wq_i/sqrt(d)) @ h ; k = wk_i @ h      [32, S]  bf16
  vT  = h.T @ wv_i.T                           [S, 32]  -> fp8e4 (+32 ones cols)
  stT = k.T @ q                                [S(t), S(s)] f32 psum (bf16 mm)
  e   = exp-approx(stT)  -> fp8e5:
          ~60% of tiles: ScalarE true exp (RNE-rounded to e5m2)
          ~40% of tiles: DVE Schraudolph bit-hack -- one tensor_scalar
          (mult+add -> int8, bitcast e5m2): e = 2^(x*log2e) via exponent bits
  oT  = (vT.T @ e) / (ones @ e)                [32, S]
        AV matmul in fp8 DoubleRow perf mode: 2 t-tiles contracted per pass at
        0.5 cyc/row -> 4x the bf16 rate. ones-cols give the softmax denominator
        in psum rows 32:63; normalize = DVE reciprocal + Pool multiply.
  out_slab = xslab + w_proj(bf16) @ oT(bf16)

Engine budget per core (cost model): exp is 33.6M elems and must be read from
PSUM, which only ScalarE (1.2 elem/ns) and DVE (0.96 elem/ns) can do; the even/
odd group split plus the unavoidable PSUM-side copies (q/k, av, vt) puts both
at ~167us busy. PE does ~109us of bf16 QK + ~27us of fp8-DR AV + ~18us misc.
Pool (no PSUM access) takes BN, the vt-ones memset and normalize multiplies.

Schedule: flat pipeline over (sg, group-of-2-t-tiles) with 3 double-buffered
2-bank score slots (exp never waits a bank refill), AV lagging 2 groups,
normalize lagging 6 (so the ScalarE av-copy never head-of-line blocks its exp
stream), BN/QKV/vT woven just-in-time into each batch's first sg, batch-0 proj
overlapped into batch-1's attention, and batch-1's proj-half0 bulk hoisted
ahead of the serial tail. Cost-model makespan 195.4us/core (ScalarE/DVE ~85%
busy; baseline bf16+ScalarE-exp kernel was 271.8us). Measured rel err 2.8e-3
vs the fp32 reference (budget 2e-2).
"""
import numpy as np
import ml_dtypes
from contextlib import ExitStack

import concourse.bass as bass
import concourse.tile as tile
from concourse import bacc, mybir
from concourse.bass_utils import run_bass_kernel_spmd

F32 = mybir.dt.float32
BF16 = mybir.dt.bfloat16
E4 = mybir.dt.float8e4
E5 = mybir.dt.float8e5
I8 = mybir.dt.int8
AF = mybir.ActivationFunctionType
ALU = mybir.AluOpType
PM = mybir.MatmulPerfMode

B, C, H, W = 2, 256, 64, 64
S = H * W          # 4096
NH, D = 8, 32      # heads, head dim
BN_EPS = 1e-5
NCORES = 8

LOG2E = 1.4426950408889634
SCH_A = 4.0 * LOG2E       # e5m2: 2 mantissa bits -> scale 2^2
SCH_B = 60.0 - 0.26       # (15 << 2) bias, C=0.26 balances bias/maxerr

_nc_cache = None


def ts(i, sz):
    return slice(i * sz, (i + 1) * sz)


def build_nc():
    nc = bacc.Bacc()
    x_d = nc.dram_tensor("x", [B, 2, 128, S], F32, kind="ExternalInput")
    xslab_d = nc.dram_tensor("xslab", [B, 2, 128, 512], F32, kind="ExternalInput")
    wqk_d = nc.dram_tensor("wqk", [128, 2, 64], BF16, kind="ExternalInput")
    wv_d = nc.dram_tensor("wv", [128, 2, 32], BF16, kind="ExternalInput")
    wproj_d = nc.dram_tensor("wproj", [32, 8, 2, 128], BF16, kind="ExternalInput")
    bnp_d = nc.dram_tensor("bnp", [128, 2, 2], F32, kind="ExternalInput")
    out_d = nc.dram_tensor("out", [B, 2, 128, 512], F32, kind="ExternalOutput")

    with tile.TileContext(nc) as tc, ExitStack() as ctx:
        const = ctx.enter_context(tc.tile_pool(name="const", bufs=1))
        xpool = ctx.enter_context(tc.tile_pool(name="xp", bufs=4))
        hpool = ctx.enter_context(tc.tile_pool(name="hp", bufs=1))
        qkpool = ctx.enter_context(tc.tile_pool(name="qk", bufs=2))
        vtpool = ctx.enter_context(tc.tile_pool(name="vt", bufs=2))
        epool = ctx.enter_context(tc.tile_pool(name="ep", bufs=2))
        opool = ctx.enter_context(tc.tile_pool(name="op", bufs=2))
        mpool = ctx.enter_context(tc.tile_pool(name="mp", bufs=2))
        pbig = ctx.enter_context(tc.tile_pool(name="pbig", bufs=3, space="PSUM"))
        pav = ctx.enter_context(tc.tile_pool(name="pav", bufs=1, space="PSUM"))
        psmall = ctx.enter_context(tc.tile_pool(name="psm", bufs=1, space="PSUM"))

        # PE p-state warmup: wide dummy matmuls that keep PE busy through the
        # ~4us input-chain latency so the first real matmuls run at full clock
        warm = const.tile([32, 512], BF16)
        nc.vector.memset(warm[:], 0.0)
        wps = psmall.tile([64, 512], F32, tag="ps", name="wps")
        for w in range(7):
            nc.tensor.matmul(wps[:], warm[0:32, 0:64], warm[:],
                             start=True, stop=True)

        # constants: bnp first on the SP queue (BN gates everything); weights
        # on the ScalarE queue, which is otherwise idle until the first exp
        bnp_sb = const.tile([128, 2, 2], F32)
        nc.sync.dma_start(bnp_sb[:], bnp_d[:])
        wqk_sb = const.tile([128, 2, 64], BF16)
        nc.scalar.dma_start(wqk_sb[:], wqk_d[:])
        wv_sb = const.tile([128, 2, 32], BF16)
        nc.scalar.dma_start(wv_sb[:], wv_d[:])
        wproj_sb = const.tile([32, 8, 2, 128], BF16)
        nc.scalar.dma_start(wproj_sb[:], wproj_d[:])

        NG = 16
        GR = [(2 * g, 2 * g + 2) for g in range(NG)]
        flat = [(sg, gi) for sg in range(8) for gi in range(NG)]
        NORM_LAG = 6    # groups between AV-stop emission and normalize ops

        def emit_proj_half(b, o_sb, half, pp, pieces, close):
            for lo, hi in pieces:
                for j in range(8):
                    nc.tensor.matmul(pp[:, lo * 64:hi * 64],
                                     wproj_sb[:, j, half, :],
                                     o_sb[:, lo:hi, j, :],
                                     start=(j == 0), stop=(j == 7))
            if close:
                xs = mpool.tile([128, 512], F32, tag="xs")
                nc.sync.dma_start(xs[:], xslab_d[b, half])
                ot = mpool.tile([128, 512], F32, tag="ot")
                nc.vector.tensor_add(ot[:], pp[:], xs[:])
                nc.sync.dma_start(out_d[b, half], ot[:])

        def emit_proj(b, o_sb):
            for half in range(2):
                pp = psmall.tile([128, 512], F32, tag="ps")
                emit_proj_half(b, o_sb, half, pp,
                               [(0, 4), (4, 7), (7, 8)], True)

        o_sbs = []
        pending = []    # (due_idx_in_next_batch, closure) carried across seam
        # BN chunk column ranges; chunk readiness gates qs/vp below
        BNC = [(0, 512), (512, 512), (1024, 1024), (2048, 1024), (3072, 1024)]
        for b in range(B):
            h_bf = hpool.tile([128, 2, S], BF16, tag="h")
            q_sb = qkpool.tile([32, S], BF16, tag="q")
            k_sb = qkpool.tile([32, S], BF16, tag="k")
            vt_sb = vtpool.tile([128, 32, 64], E4, tag="vt")
            o_sb = opool.tile([32, 8, 8, 64], BF16, tag="o")   # [d, sg, j, xx]
            e_ts, avs = {}, {}

            def emit_bn(ci, b=b, h_bf=h_bf):
                c0, cn = BNC[ci]
                for ct in range(2):
                    x_t = xpool.tile([128, S // 4], F32, tag="x")
                    nc.sync.dma_start(x_t[:, 0:cn], x_d[b, ct, :, c0:c0 + cn])
                    # batch 0's very first chunk rides DVE's idle 2x_2p
                    # window so the qs -> exp chain starts earlier
                    eng = nc.vector if (b == 0 and ci < 1) else nc.gpsimd
                    eng.tensor_scalar(
                        h_bf[:, ct, c0:c0 + cn], x_t[:, 0:cn],
                        bnp_sb[:, ct, 0:1], bnp_sb[:, ct, 1:2],
                        ALU.mult, ALU.add)

            def emit_qs(sc, h_bf=h_bf, q_sb=q_sb, k_sb=k_sb):
                qs = psmall.tile([64, 512], F32, tag="ps")
                for ct in range(2):
                    nc.tensor.matmul(qs[:], wqk_sb[:, ct, :],
                                     h_bf[:, ct, ts(sc, 512)],
                                     start=(ct == 0), stop=(ct == 1))
                nc.scalar.copy(k_sb[:, ts(sc, 512)], qs[32:64, :])
                nc.scalar.copy(q_sb[:, ts(sc, 512)], qs[0:32, :])

            def emit_vp(vg, h_bf=h_bf, vt_sb=vt_sb):
                vp = psmall.tile([128, 16, 32], F32, tag="ps", name="vp")
                for vi in range(16):
                    vtt = 16 * vg + vi
                    for ct in range(2):
                        nc.tensor.matmul(vp[:, vi, :],
                                         h_bf[:, ct, ts(vtt, 128)],
                                         wv_sb[:, ct, :],
                                         start=(ct == 0), stop=(ct == 1))
                nc.scalar.copy(vt_sb[:, ts(vg, 16), 0:32], vp[:])

            def emit_av(sg, gi, e_ts=e_ts, avs=avs, vt_sb=vt_sb):
                t0, t1 = GR[gi]
                if gi == 0:
                    avs[sg] = pav.tile([64, 512], F32, tag="av", name="av")
                nc.tensor.matmul(avs[sg][:], vt_sb[:, t0:t1, :],
                                 e_ts[sg][:, t0:t1, :],
                                 start=(gi == 0), stop=(gi == NG - 1),
                                 perf_mode=PM.DoubleRow)

            def emit_norm(sg, avs=avs, o_sb=o_sb):
                src = mpool.tile([64, 512], F32, tag="avc", name="avc")
                nc.scalar.copy(src[:], avs[sg][:])
                rc = mpool.tile([32, 512], F32, tag="rc")
                nc.vector.reciprocal(rc[:], src[32:64, :])
                o_view = o_sb[:, sg, :, :].rearrange("p j x -> p x j")
                nc.gpsimd.tensor_mul(o_view, src[0:32, :], rc[:])

            # prelude: BN/qs/vp woven just-in-time into sg0's group pipeline.
            # st(sg0, g) needs k tiles {2g, 2g+1} <= qs(floor((2g+1)/4)); each
            # qs(c) lands 2+ groups ahead of first use. vp(v) rides odd slots.
            emit_bn(0)
            emit_bn(1)
            if b == 0:
                nc.vector.memset(vt_sb[:, :, 32:64], 1.0)
            else:
                nc.gpsimd.memset(vt_sb[:, :, 32:64], 1.0)
            emit_qs(0)
            emit_qs(1)
            prelude = {
                0: [lambda: emit_bn(2)],
                1: [lambda: emit_vp(0)],
                2: [lambda: emit_qs(2)],
                4: [lambda: emit_qs(3), lambda: emit_bn(3)],
                6: [lambda: emit_qs(4), lambda: emit_bn(4)],
                8: [lambda: emit_qs(5)],
                9: [lambda: emit_vp(1)],
                10: [lambda: emit_qs(6)],
                12: [lambda: emit_qs(7)],
            }

            for idx, (sg, gi) in enumerate(flat):
                if gi == 0:
                    e_ts[sg] = epool.tile([128, 32, 512], E5, tag="e",
                                          name="e_t")
                st = pbig.tile([128, 2, 512], F32, tag="st")
                for u in range(2):
                    tt = GR[gi][0] + u
                    nc.tensor.matmul(st[:, u, :], k_sb[:, ts(tt, 128)],
                                     q_sb[:, ts(sg, 512)],
                                     start=True, stop=True)
                if gi % 2 == 0:
                    nc.scalar.activation(e_ts[sg][:, GR[gi][0]:GR[gi][1], :],
                                         st[:], AF.Exp)
                else:
                    nc.vector.tensor_scalar(
                        e_ts[sg][:, GR[gi][0]:GR[gi][1], :].bitcast(I8),
                        st[:], SCH_A, SCH_B, ALU.mult, ALU.add)
                while pending and pending[0][0] <= idx:
                    pending.pop(0)[1]()
                if sg == 0 and gi in prelude:
                    for fn in prelude[gi]:
                        fn()
                if idx > 1:
                    emit_av(*flat[idx - 2])
                # normalize for sg-1 lands NORM_LAG groups after its AV stop
                if sg > 0 and gi == NORM_LAG - 2:
                    emit_norm(sg - 1)
                # last batch: the bulk of proj half0 (needs o_sb sg 0..6)
                # streams ahead of the serial norm(sg7) -> proj tail
                if b == B - 1 and sg == 7 and gi == NORM_LAG - 1:
                    pp_h0 = psmall.tile([128, 512], F32, tag="ps")
                    emit_proj_half(b, o_sb, 0, pp_h0, [(0, 4), (4, 7)], False)
            if b == B - 1:
                emit_av(*flat[-2])
                emit_av(*flat[-1])
                emit_norm(7)
            else:
                pending[:] = [
                    (0, (lambda a=flat[-2], f=emit_av: f(*a))),
                    (1, (lambda a=flat[-1], f=emit_av: f(*a))),
                    (NORM_LAG - 2, (lambda f=emit_norm: f(7))),
                ]
                if b == 0:
                    # batch-0 proj overlaps batch-1's attention stream; by
                    # idx 24 batch-0's last normalize (Pool) has retired
                    pending.append(
                        (24, (lambda bb=b, oo=o_sb: emit_proj(bb, oo))))
            o_sbs.append(o_sb)

        # tail: close proj half0 (sg7 piece), then half1
        emit_proj_half(B - 1, o_sbs[B - 1], 0, pp_h0, [(7, 8)], True)
        pp_h1 = psmall.tile([128, 512], F32, tag="ps")
        emit_proj_half(B - 1, o_sbs[B - 1], 1, pp_h1,
                       [(0, 4), (4, 7), (7, 8)], True)

    nc.compile()
    return nc


def get_nc():
    global _nc_cache
    if _nc_cache is None:
        _nc_cache = build_nc()
    return _nc_cache


def prepare_in_maps(x, w_qkv, w_proj, gamma, beta, running_mean, running_var):
    x = np.ascontiguousarray(np.asarray(x, dtype=np.float32))
    w_qkv = np.asarray(w_qkv, dtype=np.float32)
    w_proj = np.asarray(w_proj, dtype=np.float32)
    gamma = np.asarray(gamma, dtype=np.float32)
    beta = np.asarray(beta, dtype=np.float32)
    running_mean = np.asarray(running_mean, dtype=np.float32)
    running_var = np.asarray(running_var, dtype=np.float32)

    bn_scale = gamma / np.sqrt(running_var + BN_EPS)
    bn_bias = beta - running_mean * bn_scale
    bnp = np.ascontiguousarray(
        np.stack([bn_scale.reshape(2, 128), bn_bias.reshape(2, 128)],
                 axis=-1).transpose(1, 0, 2))

    x_r = x.reshape(B, 2, 128, S)
    # w_proj^T arranged [d, j, half, o]:  wproj[d, j, half, o] = w_proj[half*128+o, j*32+d]
    wp = w_proj.reshape(2, 128, 8, 32).transpose(3, 2, 0, 1)  # [d, j, half, o]
    wp = np.ascontiguousarray(wp.astype(ml_dtypes.bfloat16))

    in_maps = []
    for i in range(NCORES):
        wq = (w_qkv[D * i:D * (i + 1)] / np.sqrt(D)).T      # [C, 32]
        wk = w_qkv[C + D * i:C + D * (i + 1)].T             # [C, 32]
        wv = w_qkv[2 * C + D * i:2 * C + D * (i + 1)].T     # [C, 32]
        wqk = np.concatenate([wq, wk], axis=1)              # [C, 64]
        wqk = np.ascontiguousarray(
            wqk.reshape(2, 128, 64).transpose(1, 0, 2).astype(ml_dtypes.bfloat16))
        wv_t = np.ascontiguousarray(
            wv.reshape(2, 128, 32).transpose(1, 0, 2).astype(ml_dtypes.bfloat16))
        xslab = np.ascontiguousarray(
            x[:, :, 8 * i:8 * (i + 1), :].reshape(B, 2, 128, 512))
        in_maps.append({
            "x": x_r, "xslab": xslab, "wqk": wqk, "wv": wv_t,
            "wproj": wp, "bnp": bnp,
        })
    return in_maps


def run(in_maps, trace=False):
    nc = get_nc()
    return run_bass_kernel_spmd(nc, in_maps, list(range(NCORES)), trace=trace)


_runner_cache = None


def get_runner():
    """Build (once) a jitted SPMD runner so repeat kernel() calls don't
    recompile. Mirrors concourse.bass2jax.run_bass_via_pjrt."""
    global _runner_cache
    if _runner_cache is not None:
        return _runner_cache
    import jax
    from jax.sharding import Mesh, PartitionSpec, NamedSharding
    from jax.experimental.shard_map import shard_map
    from concourse.bass2jax import (
        _bass_exec_p, install_neuronx_cc_hook, partition_id_tensor)

    nc = get_nc()
    install_neuronx_cc_hook()
    in_names, out_names, out_avals, zero_outs = [], [], [], []
    pname = nc.partition_id_tensor.name if nc.partition_id_tensor else None
    for alloc in nc.m.functions[0].allocations:
        if not isinstance(alloc, mybir.MemoryLocationSet):
            continue
        name = alloc.memorylocations[0].name
        if alloc.kind == "ExternalInput":
            if name != pname:
                in_names.append(name)
        elif alloc.kind == "ExternalOutput":
            out_names.append(name)
            shape = tuple(alloc.tensor_shape)
            dtype = mybir.dt.np(alloc.dtype)
            out_avals.append(jax.core.ShapedArray(shape, dtype))
            zero_outs.append(np.zeros(shape, dtype))
    n_params = len(in_names)
    all_names = list(in_names) + out_names
    if pname is not None:
        all_names.append(pname)

    def _body(*args):
        operands = list(args)
        if pname is not None:
            operands.append(partition_id_tensor())
        outs = _bass_exec_p.bind(
            *operands,
            out_avals=tuple(out_avals),
            in_names=tuple(all_names),
            out_names=tuple(out_names),
            lowering_input_output_aliases=(),
            sim_require_finite=True,
            sim_require_nnan=True,
            nc=nc,
        )
        return tuple(outs)

    devices = jax.devices()[:NCORES]
    assert len(devices) >= NCORES, f"need {NCORES} devices, got {len(devices)}"
    mesh = Mesh(np.asarray(devices), ("core",))
    nspec = NamedSharding(mesh, PartitionSpec("core"))
    fn = jax.jit(
        shard_map(_body, mesh=mesh,
                  in_specs=(PartitionSpec("core"),) * (n_params + len(out_names)),
                  out_specs=(PartitionSpec("core"),) * len(out_names),
                  check_rep=False),
        keep_unused=True,
    )
    _runner_cache = (fn, in_names, out_names, out_avals, zero_outs, nspec)
    return _runner_cache


def kernel(**inputs) -> np.ndarray:
    import jax
    fn, in_names, out_names, out_avals, zero_outs, nspec = get_runner()
    in_maps = prepare_in_maps(**inputs)
    concat_in = [
        np.concatenate([np.asarray(in_maps[c][nm]) for c in range(NCORES)],
                       axis=0)
        for nm in in_names
    ]
    concat_zeros = [np.zeros((NCORES * z.shape[0], *z.shape[1:]), z.dtype)
                    for z in zero_outs]
    dev_args = [jax.device_put(a, nspec) for a in concat_in + concat_zeros]
    res = fn(*dev_args)
    oi = out_names.index("out")
    per_core = np.asarray(res[oi]).reshape(NCORES, *out_avals[oi].shape)
    out = np.empty((B, C, H, W), np.float32)
    for i in range(NCORES):
        out[:, :, 8 * i:8 * (i + 1), :] = per_core[i].reshape(B, C, 8, W)
    return out


if __name__ == "__main__":
    rng = np.random.default_rng(0)
    ins = {
        "x": rng.standard_normal((B, C, H, W), dtype=np.float32),
        "w_qkv": rng.standard_normal((3 * C, C), dtype=np.float32) / 16.0,
        "w_proj": rng.standard_normal((C, C), dtype=np.float32) / 16.0,
        "gamma": np.ones(C, np.float32), "beta": np.zeros(C, np.float32),
        "running_mean": np.zeros(C, np.float32),
        "running_var": np.ones(C, np.float32),
    }
    print(kernel(**ins).shape)


# revision 33
# speedup vs baseline: 1.0025x; 1.0025x over previous
"""Trainium2 Bass kernel for nn_AttnBlock (B=2, C=256, H=W=64, 8 heads, d=32).

Sharding: head-parallel across 8 NeuronCores (core i <-> head i, both batches).
The reference's torch-faithful reshape h.view(B,H,W,C) folds the head dim into
the spatial rows: output rows y in [8i, 8i+8) depend ONLY on head i, so each
core computes its own 8-row output slab and the host just concatenates -- no
collectives needed.

Per-core math (S = H*W = 4096):
  h   = BN(x)                                  [C, S]   bf16
  q   = (wq_i/sqrt(d)) @ h ; k = wk_i @ h      [32, S]  bf16
  vT  = h.T @ wv_i.T                           [S, 32]  -> fp8e4 (+32 ones cols)
  stT = k.T @ q                                [S(t), S(s)] f32 psum (bf16 mm)
  e   = exp-approx(stT)  -> fp8e5:
          ~60% of tiles: ScalarE true exp (RNE-rounded to e5m2)
          ~40% of tiles: DVE Schraudolph bit-hack -- one tensor_scalar
          (mult+add -> int8, bitcast e5m2): e = 2^(x*log2e) via exponent bits
  oT  = (vT.T @ e) / (ones @ e)                [32, S]
        AV matmul in fp8 DoubleRow perf mode: 2 t-tiles contracted per pass at
        0.5 cyc/row -> 4x the bf16 rate. ones-cols give the softmax denominator
        in psum rows 32:63; normalize = DVE reciprocal + Pool multiply.
  out_slab = xslab + w_proj(bf16) @ oT(bf16)

Engine budget per core (cost model): exp is 33.6M elems and must be read from
PSUM, which only ScalarE (1.2 elem/ns) and DVE (0.96 elem/ns) can do; the even/
odd group split plus the unavoidable PSUM-side copies (q/k, av, vt) puts both
at ~167us busy. PE does ~109us of bf16 QK + ~27us of fp8-DR AV + ~18us misc.
Pool (no PSUM access) takes BN, the vt-ones memset and normalize multiplies.

Schedule: flat pipeline over (sg, group-of-2-t-tiles) with 3 double-buffered
2-bank score slots (exp never waits a bank refill), AV lagging 2 groups,
normalize lagging 6 (so the ScalarE av-copy never head-of-line blocks its exp
stream), BN/QKV/vT woven just-in-time into each batch's first sg, batch-0 proj
overlapped into batch-1's attention, and batch-1's proj-half0 bulk hoisted
ahead of the serial tail. Cost-model makespan 195.4us/core (ScalarE/DVE ~85%
busy; baseline bf16+ScalarE-exp kernel was 271.8us). Measured rel err 2.8e-3
vs the fp32 reference (budget 2e-2).
"""
import numpy as np
import ml_dtypes
from contextlib import ExitStack

import concourse.bass as bass
import concourse.tile as tile
from concourse import bacc, mybir
from concourse.bass_utils import run_bass_kernel_spmd

F32 = mybir.dt.float32
BF16 = mybir.dt.bfloat16
E4 = mybir.dt.float8e4
E5 = mybir.dt.float8e5
I8 = mybir.dt.int8
AF = mybir.ActivationFunctionType
ALU = mybir.AluOpType
PM = mybir.MatmulPerfMode

B, C, H, W = 2, 256, 64, 64
S = H * W          # 4096
NH, D = 8, 32      # heads, head dim
BN_EPS = 1e-5
NCORES = 8

LOG2E = 1.4426950408889634
SCH_A = 4.0 * LOG2E       # e5m2: 2 mantissa bits -> scale 2^2
SCH_B = 60.0 - 0.26       # (15 << 2) bias, C=0.26 balances bias/maxerr

_nc_cache = None


def ts(i, sz):
    return slice(i * sz, (i + 1) * sz)


def build_nc():
    nc = bacc.Bacc()
    x_d = nc.dram_tensor("x", [B, 2, 128, S], F32, kind="ExternalInput")
    xslab_d = nc.dram_tensor("xslab", [B, 2, 128, 512], F32, kind="ExternalInput")
    wqk_d = nc.dram_tensor("wqk", [128, 2, 64], BF16, kind="ExternalInput")
    wv_d = nc.dram_tensor("wv", [128, 2, 32], BF16, kind="ExternalInput")
    wproj_d = nc.dram_tensor("wproj", [32, 8, 2, 128], BF16, kind="ExternalInput")
    bnp_d = nc.dram_tensor("bnp", [128, 2, 2], F32, kind="ExternalInput")
    out_d = nc.dram_tensor("out", [B, 2, 128, 512], F32, kind="ExternalOutput")

    with tile.TileContext(nc) as tc, ExitStack() as ctx:
        const = ctx.enter_context(tc.tile_pool(name="const", bufs=1))
        xpool = ctx.enter_context(tc.tile_pool(name="xp", bufs=4))
        hpool = ctx.enter_context(tc.tile_pool(name="hp", bufs=1))
        qkpool = ctx.enter_context(tc.tile_pool(name="qk", bufs=2))
        vtpool = ctx.enter_context(tc.tile_pool(name="vt", bufs=2))
        epool = ctx.enter_context(tc.tile_pool(name="ep", bufs=2))
        opool = ctx.enter_context(tc.tile_pool(name="op", bufs=2))
        mpool = ctx.enter_context(tc.tile_pool(name="mp", bufs=2))
        pbig = ctx.enter_context(tc.tile_pool(name="pbig", bufs=3, space="PSUM"))
        pav = ctx.enter_context(tc.tile_pool(name="pav", bufs=1, space="PSUM"))
        psmall = ctx.enter_context(tc.tile_pool(name="psm", bufs=1, space="PSUM"))

        # PE p-state warmup: wide dummy matmuls that keep PE busy through the
        # ~4us input-chain latency so the first real matmuls run at full clock
        warm = const.tile([32, 512], BF16)
        nc.vector.memset(warm[:], 0.0)
        wps = psmall.tile([64, 512], F32, tag="ps", name="wps")
        for w in range(7):
            nc.tensor.matmul(wps[:], warm[0:32, 0:64], warm[:],
                             start=True, stop=True)

        # constants: bnp first on the SP queue (BN gates everything); weights
        # on the ScalarE queue, which is otherwise idle until the first exp
        bnp_sb = const.tile([128, 2, 2], F32)
        nc.sync.dma_start(bnp_sb[:], bnp_d[:])
        wqk_sb = const.tile([128, 2, 64], BF16)
        nc.scalar.dma_start(wqk_sb[:], wqk_d[:])
        wv_sb = const.tile([128, 2, 32], BF16)
        nc.scalar.dma_start(wv_sb[:], wv_d[:])
        wproj_sb = const.tile([32, 8, 2, 128], BF16)
        nc.scalar.dma_start(wproj_sb[:], wproj_d[:])

        NG = 16
        GR = [(2 * g, 2 * g + 2) for g in range(NG)]
        flat = [(sg, gi) for sg in range(8) for gi in range(NG)]
        NORM_LAG = 6    # groups between AV-stop emission and normalize ops

        def emit_proj_half(b, o_sb, half, pp, pieces, close):
            for lo, hi in pieces:
                for j in range(8):
                    nc.tensor.matmul(pp[:, lo * 64:hi * 64],
                                     wproj_sb[:, j, half, :],
                                     o_sb[:, lo:hi, j, :],
                                     start=(j == 0), stop=(j == 7))
            if close:
                xs = mpool.tile([128, 512], F32, tag="xs")
                nc.sync.dma_start(xs[:], xslab_d[b, half])
                ot = mpool.tile([128, 512], F32, tag="ot")
                nc.vector.tensor_add(ot[:], pp[:], xs[:])
                nc.sync.dma_start(out_d[b, half], ot[:])

        def emit_proj(b, o_sb):
            for half in range(2):
                pp = psmall.tile([128, 512], F32, tag="ps")
                emit_proj_half(b, o_sb, half, pp,
                               [(0, 4), (4, 7), (7, 8)], True)

        o_sbs = []
        pending = []    # (due_idx_in_next_batch, closure) carried across seam
        # BN chunk column ranges; chunk readiness gates qs/vp below
        BNC = [(0, 512), (512, 512), (1024, 1024), (2048, 1024), (3072, 1024)]
        for b in range(B):
            h_bf = hpool.tile([128, 2, S], BF16, tag="h")
            q_sb = qkpool.tile([32, S], BF16, tag="q")
            k_sb = qkpool.tile([32, S], BF16, tag="k")
            vt_sb = vtpool.tile([128, 32, 64], E4, tag="vt")
            o_sb = opool.tile([32, 8, 8, 64], BF16, tag="o")   # [d, sg, j, xx]
            e_ts, avs = {}, {}

            def emit_bn(ci, b=b, h_bf=h_bf):
                c0, cn = BNC[ci]
                for ct in range(2):
                    x_t = xpool.tile([128, S // 4], F32, tag="x")
                    nc.sync.dma_start(x_t[:, 0:cn], x_d[b, ct, :, c0:c0 + cn])
                    # batch 0's very first chunk rides DVE's idle 2x_2p
                    # window so the qs -> exp chain starts earlier
                    eng = nc.vector if (b == 0 and ci < 1) else nc.gpsimd
                    eng.tensor_scalar(
                        h_bf[:, ct, c0:c0 + cn], x_t[:, 0:cn],
                        bnp_sb[:, ct, 0:1], bnp_sb[:, ct, 1:2],
                        ALU.mult, ALU.add)

            def emit_qs(sc, h_bf=h_bf, q_sb=q_sb, k_sb=k_sb):
                qs = psmall.tile([64, 512], F32, tag="ps")
                for ct in range(2):
                    nc.tensor.matmul(qs[:], wqk_sb[:, ct, :],
                                     h_bf[:, ct, ts(sc, 512)],
                                     start=(ct == 0), stop=(ct == 1))
                nc.scalar.copy(k_sb[:, ts(sc, 512)], qs[32:64, :])
                nc.scalar.copy(q_sb[:, ts(sc, 512)], qs[0:32, :])

            def emit_vp(vg, h_bf=h_bf, vt_sb=vt_sb):
                vp = psmall.tile([128, 16, 32], F32, tag="ps", name="vp")
                for vi in range(16):
                    vtt = 16 * vg + vi
                    for ct in range(2):
                        nc.tensor.matmul(vp[:, vi, :],
                                         h_bf[:, ct, ts(vtt, 128)],
                                         wv_sb[:, ct, :],
                                         start=(ct == 0), stop=(ct == 1))
                nc.scalar.copy(vt_sb[:, ts(vg, 16), 0:32], vp[:])

            def emit_av(sg, gi, e_ts=e_ts, avs=avs, vt_sb=vt_sb):
                t0, t1 = GR[gi]
                if gi == 0:
                    avs[sg] = pav.tile([64, 512], F32, tag="av", name="av")
                nc.tensor.matmul(avs[sg][:], vt_sb[:, t0:t1, :],
                                 e_ts[sg][:, t0:t1, :],
                                 start=(gi == 0), stop=(gi == NG - 1),
                                 perf_mode=PM.DoubleRow)

            def emit_norm(sg, avs=avs, o_sb=o_sb):
                src = mpool.tile([64, 512], F32, tag="avc", name="avc")
                nc.scalar.copy(src[:], avs[sg][:])
                rc = mpool.tile([32, 512], F32, tag="rc")
                nc.vector.reciprocal(rc[:], src[32:64, :])
                o_view = o_sb[:, sg, :, :].rearrange("p j x -> p x j")
                nc.gpsimd.tensor_mul(o_view, src[0:32, :], rc[:])

            # prelude: BN/qs/vp woven just-in-time into sg0's group pipeline.
            # st(sg0, g) needs k tiles {2g, 2g+1} <= qs(floor((2g+1)/4)); each
            # qs(c) lands 2+ groups ahead of first use. vp(v) rides odd slots.
            emit_bn(0)
            emit_bn(1)
            if b == 0:
                nc.vector.memset(vt_sb[:, :, 32:64], 1.0)
            else:
                nc.gpsimd.memset(vt_sb[:, :, 32:64], 1.0)
            emit_qs(0)
            emit_qs(1)
            prelude = {
                0: [lambda: emit_bn(2)],
                1: [lambda: emit_vp(0)],
                2: [lambda: emit_qs(2)],
                4: [lambda: emit_qs(3), lambda: emit_bn(3)],
                6: [lambda: emit_qs(4), lambda: emit_bn(4)],
                8: [lambda: emit_qs(5)],
                9: [lambda: emit_vp(1)],
                10: [lambda: emit_qs(6)],
                12: [lambda: emit_qs(7)],
            }

            for idx, (sg, gi) in enumerate(flat):
                if gi == 0:
                    e_ts[sg] = epool.tile([128, 32, 512], E5, tag="e",
                                          name="e_t")
                st = pbig.tile([128, 2, 512], F32, tag="st")
                for u in range(2):
                    tt = GR[gi][0] + u
                    nc.tensor.matmul(st[:, u, :], k_sb[:, ts(tt, 128)],
                                     q_sb[:, ts(sg, 512)],
                                     start=True, stop=True)
                # batch 0's first sg: ScalarE is saturated by the prelude
                # q/k copies while DVE idles, so 3 even groups flip to DVE
                act_turn = gi % 2 == 0 and not (
                    b == 0 and sg == 0 and gi in (4, 8, 12))
                if act_turn:
                    nc.scalar.activation(e_ts[sg][:, GR[gi][0]:GR[gi][1], :],
                                         st[:], AF.Exp)
                else:
                    nc.vector.tensor_scalar(
                        e_ts[sg][:, GR[gi][0]:GR[gi][1], :].bitcast(I8),
                        st[:], SCH_A, SCH_B, ALU.mult, ALU.add)
                while pending and pending[0][0] <= idx:
                    pending.pop(0)[1]()
                if sg == 0 and gi in prelude:
                    for fn in prelude[gi]:
                        fn()
                if idx > 1:
                    emit_av(*flat[idx - 2])
                # normalize for sg-1 lands NORM_LAG groups after its AV stop
                if sg > 0 and gi == NORM_LAG - 2:
                    emit_norm(sg - 1)
                # last batch: the bulk of proj half0 (needs o_sb sg 0..6)
                # streams ahead of the serial norm(sg7) -> proj tail
                if b == B - 1 and sg == 7 and gi == NORM_LAG - 1:
                    pp_h0 = psmall.tile([128, 512], F32, tag="ps")
                    emit_proj_half(b, o_sb, 0, pp_h0, [(0, 4), (4, 7)], False)
            if b == B - 1:
                emit_av(*flat[-2])
                emit_av(*flat[-1])
                emit_norm(7)
            else:
                pending[:] = [
                    (0, (lambda a=flat[-2], f=emit_av: f(*a))),
                    (1, (lambda a=flat[-1], f=emit_av: f(*a))),
                    (NORM_LAG - 2, (lambda f=emit_norm: f(7))),
                ]
                if b == 0:
                    # batch-0 proj overlaps batch-1's attention stream; by
                    # idx 24 batch-0's last normalize (Pool) has retired
                    pending.append(
                        (24, (lambda bb=b, oo=o_sb: emit_proj(bb, oo))))
            o_sbs.append(o_sb)

        # tail: close proj half0 (sg7 piece), then half1
        emit_proj_half(B - 1, o_sbs[B - 1], 0, pp_h0, [(7, 8)], True)
        pp_h1 = psmall.tile([128, 512], F32, tag="ps")
        emit_proj_half(B - 1, o_sbs[B - 1], 1, pp_h1,
                       [(0, 4), (4, 7), (7, 8)], True)

    nc.compile()
    return nc


def get_nc():
    global _nc_cache
    if _nc_cache is None:
        _nc_cache = build_nc()
    return _nc_cache


def prepare_in_maps(x, w_qkv, w_proj, gamma, beta, running_mean, running_var):
    x = np.ascontiguousarray(np.asarray(x, dtype=np.float32))
    w_qkv = np.asarray(w_qkv, dtype=np.float32)
    w_proj = np.asarray(w_proj, dtype=np.float32)
    gamma = np.asarray(gamma, dtype=np.float32)
    beta = np.asarray(beta, dtype=np.float32)
    running_mean = np.asarray(running_mean, dtype=np.float32)
    running_var = np.asarray(running_var, dtype=np.float32)

    bn_scale = gamma / np.sqrt(running_var + BN_EPS)
    bn_bias = beta - running_mean * bn_scale
    bnp = np.ascontiguousarray(
        np.stack([bn_scale.reshape(2, 128), bn_bias.reshape(2, 128)],
                 axis=-1).transpose(1, 0, 2))

    x_r = x.reshape(B, 2, 128, S)
    # w_proj^T arranged [d, j, half, o]:  wproj[d, j, half, o] = w_proj[half*128+o, j*32+d]
    wp = w_proj.reshape(2, 128, 8, 32).transpose(3, 2, 0, 1)  # [d, j, half, o]
    wp = np.ascontiguousarray(wp.astype(ml_dtypes.bfloat16))

    in_maps = []
    for i in range(NCORES):
        wq = (w_qkv[D * i:D * (i + 1)] / np.sqrt(D)).T      # [C, 32]
        wk = w_qkv[C + D * i:C + D * (i + 1)].T             # [C, 32]
        wv = w_qkv[2 * C + D * i:2 * C + D * (i + 1)].T     # [C, 32]
        wqk = np.concatenate([wq, wk], axis=1)              # [C, 64]
        wqk = np.ascontiguousarray(
            wqk.reshape(2, 128, 64).transpose(1, 0, 2).astype(ml_dtypes.bfloat16))
        wv_t = np.ascontiguousarray(
            wv.reshape(2, 128, 32).transpose(1, 0, 2).astype(ml_dtypes.bfloat16))
        xslab = np.ascontiguousarray(
            x[:, :, 8 * i:8 * (i + 1), :].reshape(B, 2, 128, 512))
        in_maps.append({
            "x": x_r, "xslab": xslab, "wqk": wqk, "wv": wv_t,
            "wproj": wp, "bnp": bnp,
        })
    return in_maps


def run(in_maps, trace=False):
    nc = get_nc()
    return run_bass_kernel_spmd(nc, in_maps, list(range(NCORES)), trace=trace)


_runner_cache = None


def get_runner():
    """Build (once) a jitted SPMD runner so repeat kernel() calls don't
    recompile. Mirrors concourse.bass2jax.run_bass_via_pjrt."""
    global _runner_cache
    if _runner_cache is not None:
        return _runner_cache
    import jax
    from jax.sharding import Mesh, PartitionSpec, NamedSharding
    from jax.experimental.shard_map import shard_map
    from concourse.bass2jax import (
        _bass_exec_p, install_neuronx_cc_hook, partition_id_tensor)

    nc = get_nc()
    install_neuronx_cc_hook()
    in_names, out_names, out_avals, zero_outs = [], [], [], []
    pname = nc.partition_id_tensor.name if nc.partition_id_tensor else None
    for alloc in nc.m.functions[0].allocations:
        if not isinstance(alloc, mybir.MemoryLocationSet):
            continue
        name = alloc.memorylocations[0].name
        if alloc.kind == "ExternalInput":
            if name != pname:
                in_names.append(name)
        elif alloc.kind == "ExternalOutput":
            out_names.append(name)
            shape = tuple(alloc.tensor_shape)
            dtype = mybir.dt.np(alloc.dtype)
            out_avals.append(jax.core.ShapedArray(shape, dtype))
            zero_outs.append(np.zeros(shape, dtype))
    n_params = len(in_names)
    all_names = list(in_names) + out_names
    if pname is not None:
        all_names.append(pname)

    def _body(*args):
        operands = list(args)
        if pname is not None:
            operands.append(partition_id_tensor())
        outs = _bass_exec_p.bind(
            *operands,
            out_avals=tuple(out_avals),
            in_names=tuple(all_names),
            out_names=tuple(out_names),
            lowering_input_output_aliases=(),
            sim_require_finite=True,
            sim_require_nnan=True,
            nc=nc,
        )
        return tuple(outs)

    devices = jax.devices()[:NCORES]
    assert len(devices) >= NCORES, f"need {NCORES} devices, got {len(devices)}"
    mesh = Mesh(np.asarray(devices), ("core",))
    nspec = NamedSharding(mesh, PartitionSpec("core"))
    fn = jax.jit(
        shard_map(_body, mesh=mesh,
                  in_specs=(PartitionSpec("core"),) * (n_params + len(out_names)),
                  out_specs=(PartitionSpec("core"),) * len(out_names),
                  check_rep=False),
        keep_unused=True,
    )
    _runner_cache = (fn, in_names, out_names, out_avals, zero_outs, nspec)
    return _runner_cache


def kernel(**inputs) -> np.ndarray:
    import jax
    fn, in_names, out_names, out_avals, zero_outs, nspec = get_runner()
    in_maps = prepare_in_maps(**inputs)
    concat_in = [
        np.concatenate([np.asarray(in_maps[c][nm]) for c in range(NCORES)],
                       axis=0)
        for nm in in_names
    ]
    concat_zeros = [np.zeros((NCORES * z.shape[0], *z.shape[1:]), z.dtype)
                    for z in zero_outs]
    dev_args = [jax.device_put(a, nspec) for a in concat_in + concat_zeros]
    res = fn(*dev_args)
    oi = out_names.index("out")
    per_core = np.asarray(res[oi]).reshape(NCORES, *out_avals[oi].shape)
    out = np.empty((B, C, H, W), np.float32)
    for i in range(NCORES):
        out[:, :, 8 * i:8 * (i + 1), :] = per_core[i].reshape(B, C, 8, W)
    return out


if __name__ == "__main__":
    rng = np.random.default_rng(0)
    ins = {
        "x": rng.standard_normal((B, C, H, W), dtype=np.float32),
        "w_qkv": rng.standard_normal((3 * C, C), dtype=np.float32) / 16.0,
        "w_proj": rng.standard_normal((C, C), dtype=np.float32) / 16.0,
        "gamma": np.ones(C, np.float32), "beta": np.zeros(C, np.float32),
        "running_mean": np.zeros(C, np.float32),
        "running_var": np.ones(C, np.float32),
    }
    print(kernel(**ins).shape)
